# revision 32
# baseline (speedup 1.0000x reference)
"""NoisyNet dense layer (training mode) on 8 TRN2 NeuronCores.

out[b,u] = x @ W_mu + eps_out * ((x*eps_in) @ W_sigma) + bias_mu + bias_sigma*eps_out

Sharding: data-parallel over batch (4096 -> 512 rows/core), weights/biases
replicated. On-device math runs in a transposed layout ([D,B]/[U,B]) so the
contraction dim D lands on SBUF partitions; the host does the (free)
transposes, dtype casts and the final gather.

Two-phase schedule: phase 1 runs all 16 mean-term k-loops in bf16 (PSUM
drains to SBUF through the bias-add ACT op), phase 2 runs the noise-term
k-loops and the elementwise combine. In phase 2, k-tiles KB8..15 of the
contraction run as fp8e4 DoubleRow matmuls (2 k-tiles per pass at
2 MACs/cell/cycle); W_sigma is pre-scaled by 512 on the host (exact in
bf16) so both halves share one PSUM scale, undone by the epilogue ACT's
scale=1/512. Error: fp8 on 10/16 of the noise contraction gives rel err
~1.83e-2 vs the 2e-2 gate (verified exactly against the reference on
host; HW matched the host prediction to 6 digits at the 8/16 split).

Start: the x stream owns the early DMA bandwidth (eps_in/eps_out issues
are paced behind phase-1 progress on the scalar queue), u=0 and u=1
k-loops interleave over arriving x chunks so the PE does double work
during the DMA-bound window, and warm-up matmuls on zeroed SBUF keep the
PE HAM-busy (full 2.4 GHz clock) until real data lands.
"""

import numpy as np
import ml_dtypes

import concourse.bacc as bacc
import concourse.mybir as mybir
import concourse.tile as tile
from concourse.bass_utils import run_bass_kernel_spmd

N_CORES = 8
B, D, U = 4096, 2048, 2048
BL = B // N_CORES          # 512 batch rows per core
P = 128                    # partitions
KT = D // P                # 16 contraction tiles
UT = U // P                # 16 output tiles
KC = 4                     # k-tiles per activation DMA chunk
NCH = KT // KC             # 4 chunks
KB8 = 6                    # noise-term k-tiles done in bf16 (k 0..5)
NDR = (KT - KB8) // 2      # fp8 DoubleRow passes (k 6..15 as 5 pairs)
SW = 512.0                 # weight scale for the noise matmul (exact pow2)
WARM_MMS = 8               # warm-up matmuls to ride out the DMA ramp
WARM_N = 512               # free dim of warm-up matmuls
BF16 = mybir.dt.bfloat16
FP32 = mybir.dt.float32
FP8 = mybir.dt.float8e4

_NBF = ml_dtypes.bfloat16
_NF8 = ml_dtypes.float8_e4m3

_cached = None


def _build():
    nc = bacc.Bacc("TRN2", target_bir_lowering=False, debug=False)

    # activations laid out [P, KT, BL]: partition p holds d = k*128+p
    xT = nc.declare_dram_parameter("xT", [P, KT, BL], BF16, isOutput=False)
    eiT = nc.declare_dram_parameter("eiT", [P, KT, BL], BF16, isOutput=False)
    eoT = nc.declare_dram_parameter("eoT", [P, UT, BL], BF16, isOutput=False)
    wmu = nc.declare_dram_parameter("wmu", [UT, P, KT * P], BF16, isOutput=False)
    wsgb = nc.declare_dram_parameter("wsgb", [UT, P, KB8 * P], BF16, isOutput=False)
    wsg8 = nc.declare_dram_parameter("wsg8", [UT, P, NDR * 2 * P], FP8, isOutput=False)
    bmu = nc.declare_dram_parameter("bmu", [P, UT], FP32, isOutput=False)
    bsg = nc.declare_dram_parameter("bsg", [P, UT], FP32, isOutput=False)
    outT = nc.declare_dram_parameter("outT", [UT, P, BL], FP32, isOutput=True)

    with tile.TileContext(nc) as tc:
        with (
            tc.tile_pool(name="acts", bufs=1) as acts,
            tc.tile_pool(name="w", bufs=8) as wp,
            tc.tile_pool(name="ws", bufs=4) as wsp,
            tc.tile_pool(name="bias", bufs=1) as bp,
            tc.tile_pool(name="psum", bufs=4, space="PSUM") as pp,
            tc.tile_pool(name="psumn", bufs=3, space="PSUM") as ppn,
            tc.tile_pool(name="mean", bufs=UT) as mp,
            tc.tile_pool(name="tmp", bufs=2) as tp,
            tc.tile_pool(name="out", bufs=3) as op,
        ):
            # HAM warm-up: matmuls on zeroed SBUF during the initial DMA wait
            # so the real matmuls run at 2.4 GHz from the start.
            warm_in = bp.tile([P, BL], BF16, tag="warmin")
            nc.gpsimd.memset(warm_in[:], 0.0)
            warm_ps = ppn.tile([P, BL], FP32, tag="psn")
            for _ in range(WARM_MMS):
                nc.tensor.matmul(warm_ps[:, :WARM_N], warm_in[:, :P],
                                 warm_in[:, :WARM_N])

            # Weight stream (sync queue): all W_mu first, then all W_sigma.
            # Early fetches split so the first matmuls can start sooner.
            wm_tiles = {}
            ws_tiles = {}

            def fetch_wm(u, split=False):
                wm = wp.tile([P, KT * P], BF16, tag="wm")
                if split:
                    nc.sync.dma_start(wm[:, :KC * P], wmu[u][:, :KC * P])
                    nc.sync.dma_start(wm[:, KC * P:], wmu[u][:, KC * P:])
                else:
                    nc.sync.dma_start(wm[:], wmu[u])
                wm_tiles[u] = wm

            def fetch_ws(u):
                wsb_t = wsp.tile([P, KB8 * P], BF16, tag="wsb")
                nc.sync.dma_start(wsb_t[:], wsgb[u])
                ws8_t = wsp.tile([P, NDR, 2, P], FP8, tag="ws8")
                nc.sync.dma_start(ws8_t[:], wsg8[u])
                ws_tiles[u] = (wsb_t, ws8_t)

            fetch_wm(0, split=True)
            fetch_wm(1, split=True)
            fetch_wm(2, split=True)
            fetch_wm(3)

            # x stream (scalar queue) gets the full early DMA bandwidth;
            # eps_in / eps_out issues are paced into the phase-1 loop below.
            x_sb = acts.tile([P, KT, BL], BF16, tag="x")
            ei_sb = acts.tile([P, KT, BL], BF16, tag="ei")
            zb_sb = acts.tile([P, KB8, BL], BF16, tag="zb")
            z8_sb = acts.tile([P, NDR, 2, BL], FP8, tag="z8")
            eo_sb = acts.tile([P, UT, BL], BF16, tag="eo")

            # The scalar queue serves its DMAs in issue order, so eps_in /
            # eps_out queued behind x can never slow the x stream down; and
            # issuing them unconditionally (not paced on compute progress)
            # keeps the shared DMA-semaphore rotation free of cross-queue
            # dependency cycles with the weight stream.
            nc.scalar.dma_start(x_sb[:, 0:1, :], xT[:, 0:1, :])
            nc.scalar.dma_start(x_sb[:, 1:KC, :], xT[:, 1:KC, :])
            for c in range(1, NCH):
                s = slice(c * KC, (c + 1) * KC)
                nc.scalar.dma_start(x_sb[:, s, :], xT[:, s, :])
            for p in range(KT // 2):
                s = slice(2 * p, 2 * p + 2)
                nc.scalar.dma_start(ei_sb[:, s, :], eiT[:, s, :])
            for p in range(UT // 2):
                s = slice(2 * p, 2 * p + 2)
                nc.scalar.dma_start(eo_sb[:, s, :], eoT[:, s, :])

            def z_mult(p):
                # z production for ei piece p; emitted into the phase-1 loop
                # AFTER that piece has landed, so it never blocks the vector
                # queue's FIFO ahead of the mean-term PSUM drains.
                s = slice(2 * p, 2 * p + 2)
                if 2 * p < KB8:
                    nc.vector.tensor_mul(zb_sb[:, s, :], x_sb[:, s, :],
                                         ei_sb[:, s, :])
                else:
                    j = (2 * p - KB8) // 2
                    nc.vector.tensor_mul(z8_sb[:, j], x_sb[:, s, :],
                                         ei_sb[:, s, :])

            # biases (tiny) on the gpsimd SWDGE queue, early.
            bmu_t = bp.tile([P, UT], FP32, tag="bmu")
            nc.gpsimd.dma_start(bmu_t[:], bmu[:])
            bsg_t = bp.tile([P, UT], FP32, tag="bsg")
            nc.gpsimd.dma_start(bsg_t[:], bsg[:])

            # ---- Phase 1: mean terms. t_m[u] = W_mu[u].T @ x + bias_mu[u] ----
            t_m = []

            def drain_mean(u, pm):
                # drain on the vector engine: during phase 1 the scalar queue
                # is a wall of DMA issues whose semaphore-rotation guards
                # resolve only at transfer pace — an ACT behind them would
                # stall the PE when the PSUM pool wraps.
                tm = mp.tile([P, BL], FP32, tag="tm", name=f"tm{u}")
                nc.vector.tensor_scalar_add(tm[:], pm[:], bmu_t[:, u:u + 1])
                t_m.append(tm)

            # u=0..2 interleaved over arriving x chunks: the PE does three
            # k-loops chunk-by-chunk while x streams in (u2 lags one chunk
            # since its weights arrive behind wm0/wm1 on the sync queue).
            pms = [pp.tile([P, BL], FP32, tag="psm", name=f"pm{i}")
                   for i in range(3)]
            wms = [wm_tiles.pop(u) for u in range(3)]

            def ileave_mm(u, k):
                nc.tensor.matmul(
                    pms[u][:], wms[u][:, k * P:(k + 1) * P], x_sb[:, k, :],
                    start=(k == 0), stop=(k == KT - 1),
                )

            for c in range(NCH):
                for u in (0, 1):
                    for k in range(c * KC, (c + 1) * KC):
                        ileave_mm(u, k)
                if c >= 1:
                    for k in range((c - 1) * KC, c * KC):
                        ileave_mm(2, k)
            for k in range((NCH - 1) * KC, KT):
                ileave_mm(2, k)
            for u in range(3):
                drain_mean(u, pms[u])

            for u in range(3, UT):
                if u + 1 < UT:
                    fetch_wm(u + 1)
                if u >= UT - 3:
                    fetch_ws(u - (UT - 3))
                wm_t = wm_tiles.pop(u)
                pm = pp.tile([P, BL], FP32, tag="psm")
                for k in range(KT):
                    nc.tensor.matmul(
                        pm[:], wm_t[:, k * P:(k + 1) * P], x_sb[:, k, :],
                        start=(k == 0), stop=(k == KT - 1),
                    )
                drain_mean(u, pm)
                if 3 <= u <= 10:
                    z_mult(u - 3)

            # ---- Phase 2: noise terms + combine ----
            # PSUM holds 512*noise (both weight halves pre-scaled by 512);
            # the ACT drain applies scale=1/512 and the bias_sigma add.
            for u in range(UT):
                un = u + 3
                if 3 <= un < UT:
                    fetch_ws(un)
                wsb_t, ws8_t = ws_tiles.pop(u)
                # last tiles: split batch so the epilogue pipelines with the
                # final matmuls instead of serializing after them.
                halves = (0, BL // 2, BL) if u >= UT - 2 else (0, BL)
                for h in range(len(halves) - 1):
                    lo, hi = halves[h], halves[h + 1]
                    pn = ppn.tile([P, hi - lo], FP32, tag="psn")
                    for k in range(KB8):
                        nc.tensor.matmul(
                            pn[:], wsb_t[:, k * P:(k + 1) * P], zb_sb[:, k, lo:hi],
                            start=(k == 0), stop=False,
                        )
                    for j in range(NDR):
                        nc.tensor.matmul(
                            pn[:], ws8_t[:, j], z8_sb[:, j, :, lo:hi],
                            start=False, stop=(j == NDR - 1),
                            perf_mode=mybir.MatmulPerfMode.DoubleRow,
                        )
                    t_n = tp.tile([P, hi - lo], FP32, tag="tn")
                    nc.scalar.activation(
                        t_n[:], pn[:], mybir.ActivationFunctionType.Identity,
                        bias=bsg_t[:, u:u + 1], scale=1.0 / SW,
                    )
                    pr = tp.tile([P, hi - lo], FP32, tag="pr")
                    nc.vector.tensor_mul(pr[:], t_n[:], eo_sb[:, u, lo:hi])
                    o = op.tile([P, hi - lo], FP32, tag="o")
                    nc.vector.tensor_add(o[:], pr[:], t_m[u][:, lo:hi])
                    # out rides the sync HWDGE queue (idle by the tail),
                    # avoiding the slow SWDGE end-of-kernel drain.
                    nc.sync.dma_start(outT[u][:, lo:hi], o[:])

    nc.compile()
    return nc


def _get_nc():
    global _cached
    if _cached is None:
        _cached = _build()
    return _cached


def kernel(x, weight_mu, weight_sigma, bias_mu, bias_sigma, eps_in, eps_out,
           _trace=False):
    nc = _get_nc()

    # Host-side layout prep (transposes + dtype casts only; no layer math).
    def to_pkb(a):  # [B, D] -> per-core [P, KT, BL] (partition p holds k*128+p)
        a = np.ascontiguousarray(a.astype(_NBF))
        return [
            np.ascontiguousarray(
                a[c * BL:(c + 1) * BL].T.reshape(KT, P, BL).transpose(1, 0, 2))
            for c in range(N_CORES)
        ]

    xs = to_pkb(x)
    eis = to_pkb(eps_in)
    eos = to_pkb(eps_out)  # same transform, u in place of k

    def w_blocks(w, kt):  # [kt*P, U] -> [UT, P(d within block), kt*P] bf16
        wb = w.astype(_NBF).reshape(kt, P, UT, P).transpose(2, 1, 0, 3)
        return np.ascontiguousarray(wb.reshape(UT, P, kt * P))

    wmu_h = w_blocks(weight_mu, KT)
    wsg_s = (weight_sigma.astype(np.float32) * SW)
    wsgb_h = w_blocks(wsg_s[:KB8 * P], KB8)
    # [NDR*2*P, U] -> [UT, P, NDR, 2, P] fp8 (pair i of DoubleRow pass j
    # holds d = (KB8+2j+i)*128 + p)
    w8 = wsg_s[KB8 * P:].reshape(NDR, 2, P, UT, P).transpose(3, 2, 0, 1, 4)
    wsg8_h = np.ascontiguousarray(
        w8.reshape(UT, P, NDR * 2 * P).astype(_NF8))
    bmu_h = np.ascontiguousarray(bias_mu.astype(np.float32).reshape(UT, P).T)
    bsg_h = np.ascontiguousarray(bias_sigma.astype(np.float32).reshape(UT, P).T)

    in_maps = [
        {
            "xT": xs[c],
            "eiT": eis[c],
            "eoT": eos[c],
            "wmu": wmu_h,
            "wsgb": wsgb_h,
            "wsg8": wsg8_h,
            "bmu": bmu_h,
            "bsg": bsg_h,
        }
        for c in range(N_CORES)
    ]

    res = run_bass_kernel_spmd(nc, in_maps, core_ids=list(range(N_CORES)),
                               trace=_trace)
    kernel.last_result = res

    out = np.empty((B, U), dtype=np.float32)
    for c in range(N_CORES):
        oc = res.results[c]["outT"]  # [UT, P, BL]
        out[c * BL:(c + 1) * BL] = oc.transpose(2, 0, 1).reshape(BL, U)
    return out


# revision 33
# speedup vs baseline: 1.1863x; 1.1863x over previous
"""NoisyNet dense layer (training mode) on 8 TRN2 NeuronCores.

out[b,u] = x @ W_mu + eps_out * ((x*eps_in) @ W_sigma) + bias_mu + bias_sigma*eps_out

Sharding: data-parallel over batch (4096 -> 512 rows/core), weights/biases
replicated. On-device math runs in a transposed layout ([D,B]/[U,B]) so the
contraction dim D lands on SBUF partitions; the host does the (free)
transposes, dtype casts and the final gather.

Two-phase schedule: phase 1 runs all 16 mean-term k-loops in bf16 (PSUM
drains to SBUF through the bias-add ACT op), phase 2 runs the noise-term
k-loops and the elementwise combine. In phase 2, k-tiles KB8..15 of the
contraction run as fp8e4 DoubleRow matmuls (2 k-tiles per pass at
2 MACs/cell/cycle); W_sigma is pre-scaled by 512 on the host (exact in
bf16) so both halves share one PSUM scale, undone by the epilogue ACT's
scale=1/512. Error: fp8 on 10/16 of the noise contraction gives rel err
~1.83e-2 vs the 2e-2 gate (verified exactly against the reference on
host; HW matched the host prediction to 6 digits at the 8/16 split).

Scheduling notes (hard-won against the trace):
- DMA bandwidth is shared roughly per-QUEUE, so the weight stream (sync
  queue) can prefetch deep (bufs=8) without slowing the x stream
  (scalar queue) at all.
- Per-queue DMAs are served in issue order, so eps_in/eps_out queued
  behind x can never delay x; issuing them unconditionally (never paced
  on compute progress) keeps the shared DMA-semaphore rotation free of
  cross-queue dependency cycles with the weight stream.
- Phase-1 PSUM drains run on the vector engine: during phase 1 the
  scalar queue is a wall of DMA issues whose semaphore-rotation guards
  resolve only at transfer pace, and an ACT queued behind them stalls
  the PE when the PSUM pool wraps.  The z-production multiplies are
  emitted into the phase-1 loop after their eps_in pieces land so they
  never block the vector queue ahead of those drains.
- u=0..2 k-loops interleave over arriving x chunks so the PE does
  triple work during the DMA-bound start window; warm-up matmuls on
  zeroed SBUF flip the HAM clock gate to 2.4 GHz before real work.
- Outputs ride the sync HWDGE queue (idle by the tail); gpsimd SWDGE
  carries only the tiny biases (mid-kernel block-boundary DRAINs flush
  SWDGE, so nothing long-lived may be outstanding there).
"""

import numpy as np
import ml_dtypes

import concourse.bacc as bacc
import concourse.mybir as mybir
import concourse.tile as tile
from concourse.bass_utils import run_bass_kernel_spmd

N_CORES = 8
B, D, U = 4096, 2048, 2048
BL = B // N_CORES          # 512 batch rows per core
P = 128                    # partitions
KT = D // P                # 16 contraction tiles
UT = U // P                # 16 output tiles
KC = 4                     # k-tiles per activation DMA chunk
NCH = KT // KC             # 4 chunks
KB8 = 6                    # noise-term k-tiles done in bf16 (k 0..5)
NDR = (KT - KB8) // 2      # fp8 DoubleRow passes (k 6..15 as 5 pairs)
SW = 512.0                 # weight scale for the noise matmul (exact pow2)
WARM_MMS = 8               # warm-up matmuls to ride out the DMA ramp
WARM_N = 512               # free dim of warm-up matmuls
BF16 = mybir.dt.bfloat16
FP32 = mybir.dt.float32
FP8 = mybir.dt.float8e4

_NBF = ml_dtypes.bfloat16
_NF8 = ml_dtypes.float8_e4m3

_cached = None


def _build():
    nc = bacc.Bacc("TRN2", target_bir_lowering=False, debug=False)

    # activations laid out [P, KT, BL]: partition p holds d = k*128+p
    xT = nc.declare_dram_parameter("xT", [P, KT, BL], BF16, isOutput=False)
    eiT = nc.declare_dram_parameter("eiT", [P, KT, BL], BF16, isOutput=False)
    eoT = nc.declare_dram_parameter("eoT", [P, UT, BL], BF16, isOutput=False)
    wmu = nc.declare_dram_parameter("wmu", [UT, P, KT * P], BF16, isOutput=False)
    wsgb = nc.declare_dram_parameter("wsgb", [UT, P, KB8 * P], BF16, isOutput=False)
    wsg8 = nc.declare_dram_parameter("wsg8", [UT, P, NDR * 2 * P], FP8, isOutput=False)
    bmu = nc.declare_dram_parameter("bmu", [P, UT], FP32, isOutput=False)
    bsg = nc.declare_dram_parameter("bsg", [P, UT], FP32, isOutput=False)
    outT = nc.declare_dram_parameter("outT", [UT, P, BL], FP32, isOutput=True)

    with tile.TileContext(nc) as tc:
        with (
            tc.tile_pool(name="acts", bufs=1) as acts,
            tc.tile_pool(name="w", bufs=8) as wp,
            tc.tile_pool(name="ws", bufs=4) as wsp,
            tc.tile_pool(name="bias", bufs=1) as bp,
            tc.tile_pool(name="psum", bufs=4, space="PSUM") as pp,
            tc.tile_pool(name="psumn", bufs=3, space="PSUM") as ppn,
            tc.tile_pool(name="mean", bufs=UT) as mp,
            tc.tile_pool(name="tmp", bufs=2) as tp,
            tc.tile_pool(name="out", bufs=3) as op,
        ):
            # HAM warm-up: matmuls on zeroed SBUF during the initial DMA wait
            # so the real matmuls run at 2.4 GHz from the start.
            warm_in = bp.tile([P, BL], BF16, tag="warmin")
            nc.gpsimd.memset(warm_in[:], 0.0)
            warm_ps = ppn.tile([P, BL], FP32, tag="psn")
            for _ in range(WARM_MMS):
                nc.tensor.matmul(warm_ps[:, :WARM_N], warm_in[:, :P],
                                 warm_in[:, :WARM_N])

            # Weight stream (sync queue): all W_mu first, then all W_sigma.
            # Early fetches split so the first matmuls can start sooner.
            wm_tiles = {}
            ws_tiles = {}

            def fetch_wm(u, split=False):
                wm = wp.tile([P, KT * P], BF16, tag="wm")
                if split:
                    nc.sync.dma_start(wm[:, :KC * P], wmu[u][:, :KC * P])
                    nc.sync.dma_start(wm[:, KC * P:], wmu[u][:, KC * P:])
                else:
                    nc.sync.dma_start(wm[:], wmu[u])
                wm_tiles[u] = wm

            def fetch_ws(u):
                wsb_t = wsp.tile([P, KB8 * P], BF16, tag="wsb")
                nc.sync.dma_start(wsb_t[:], wsgb[u])
                ws8_t = wsp.tile([P, NDR, 2, P], FP8, tag="ws8")
                nc.sync.dma_start(ws8_t[:], wsg8[u])
                ws_tiles[u] = (wsb_t, ws8_t)

            fetch_wm(0, split=True)
            fetch_wm(1, split=True)
            fetch_wm(2, split=True)
            fetch_wm(3)

            # x stream (scalar queue) gets the full early DMA bandwidth;
            # eps_in / eps_out issues are paced into the phase-1 loop below.
            x_sb = acts.tile([P, KT, BL], BF16, tag="x")
            ei_sb = acts.tile([P, KT, BL], BF16, tag="ei")
            zb_sb = acts.tile([P, KB8, BL], BF16, tag="zb")
            z8_sb = acts.tile([P, NDR, 2, BL], FP8, tag="z8")
            eo_sb = acts.tile([P, UT, BL], BF16, tag="eo")

            # The scalar queue serves its DMAs in issue order, so eps_in /
            # eps_out queued behind x can never slow the x stream down; and
            # issuing them unconditionally (not paced on compute progress)
            # keeps the shared DMA-semaphore rotation free of cross-queue
            # dependency cycles with the weight stream.
            nc.scalar.dma_start(x_sb[:, 0:1, :], xT[:, 0:1, :])
            nc.scalar.dma_start(x_sb[:, 1:KC, :], xT[:, 1:KC, :])
            for c in range(1, NCH):
                s = slice(c * KC, (c + 1) * KC)
                nc.scalar.dma_start(x_sb[:, s, :], xT[:, s, :])
            for p in range(KT // 2):
                s = slice(2 * p, 2 * p + 2)
                nc.scalar.dma_start(ei_sb[:, s, :], eiT[:, s, :])
            for p in range(UT // 2):
                s = slice(2 * p, 2 * p + 2)
                nc.scalar.dma_start(eo_sb[:, s, :], eoT[:, s, :])

            def z_mult(p):
                # z production for ei piece p; emitted into the phase-1 loop
                # AFTER that piece has landed, so it never blocks the vector
                # queue's FIFO ahead of the mean-term PSUM drains.
                s = slice(2 * p, 2 * p + 2)
                if 2 * p < KB8:
                    nc.vector.tensor_mul(zb_sb[:, s, :], x_sb[:, s, :],
                                         ei_sb[:, s, :])
                else:
                    j = (2 * p - KB8) // 2
                    nc.vector.tensor_mul(z8_sb[:, j], x_sb[:, s, :],
                                         ei_sb[:, s, :])

            # biases (tiny) on the gpsimd SWDGE queue, early.
            bmu_t = bp.tile([P, UT], FP32, tag="bmu")
            nc.gpsimd.dma_start(bmu_t[:], bmu[:])
            bsg_t = bp.tile([P, UT], FP32, tag="bsg")
            nc.gpsimd.dma_start(bsg_t[:], bsg[:])

            # ---- Phase 1: mean terms. t_m[u] = W_mu[u].T @ x + bias_mu[u] ----
            t_m = []

            def drain_mean(u, pm):
                # drain on the vector engine: during phase 1 the scalar queue
                # is a wall of DMA issues whose semaphore-rotation guards
                # resolve only at transfer pace — an ACT behind them would
                # stall the PE when the PSUM pool wraps.
                tm = mp.tile([P, BL], FP32, tag="tm", name=f"tm{u}")
                nc.vector.tensor_scalar_add(tm[:], pm[:], bmu_t[:, u:u + 1])
                t_m.append(tm)

            # u=0..2 interleaved over arriving x chunks: the PE does three
            # k-loops chunk-by-chunk while x streams in (u2 lags one chunk
            # since its weights arrive behind wm0/wm1 on the sync queue).
            pms = [pp.tile([P, BL], FP32, tag="psm", name=f"pm{i}")
                   for i in range(3)]
            wms = [wm_tiles.pop(u) for u in range(3)]

            def ileave_mm(u, k):
                nc.tensor.matmul(
                    pms[u][:], wms[u][:, k * P:(k + 1) * P], x_sb[:, k, :],
                    start=(k == 0), stop=(k == KT - 1),
                )

            for c in range(NCH):
                for u in (0, 1):
                    for k in range(c * KC, (c + 1) * KC):
                        ileave_mm(u, k)
                if c >= 1:
                    for k in range((c - 1) * KC, c * KC):
                        ileave_mm(2, k)
            for k in range((NCH - 1) * KC, KT):
                ileave_mm(2, k)
            for u in range(3):
                drain_mean(u, pms[u])

            for u in range(3, UT):
                if u + 1 < UT:
                    fetch_wm(u + 1)
                if u >= UT - 3:
                    fetch_ws(u - (UT - 3))
                wm_t = wm_tiles.pop(u)
                pm = pp.tile([P, BL], FP32, tag="psm")
                for k in range(KT):
                    nc.tensor.matmul(
                        pm[:], wm_t[:, k * P:(k + 1) * P], x_sb[:, k, :],
                        start=(k == 0), stop=(k == KT - 1),
                    )
                drain_mean(u, pm)
                if 3 <= u <= 10:
                    z_mult(u - 3)

            # ---- Phase 2: noise terms + combine ----
            # PSUM holds 512*noise (both weight halves pre-scaled by 512);
            # the ACT drain applies scale=1/512 and the bias_sigma add.
            for u in range(UT):
                un = u + 3
                if 3 <= un < UT:
                    fetch_ws(un)
                wsb_t, ws8_t = ws_tiles.pop(u)
                # last tiles: split batch so the epilogue pipelines with the
                # final matmuls instead of serializing after them.
                halves = (0, BL // 2, BL) if u >= UT - 2 else (0, BL)
                for h in range(len(halves) - 1):
                    lo, hi = halves[h], halves[h + 1]
                    pn = ppn.tile([P, hi - lo], FP32, tag="psn")
                    for k in range(KB8):
                        nc.tensor.matmul(
                            pn[:], wsb_t[:, k * P:(k + 1) * P], zb_sb[:, k, lo:hi],
                            start=(k == 0), stop=False,
                        )
                    for j in range(NDR):
                        nc.tensor.matmul(
                            pn[:], ws8_t[:, j], z8_sb[:, j, :, lo:hi],
                            start=False, stop=(j == NDR - 1),
                            perf_mode=mybir.MatmulPerfMode.DoubleRow,
                        )
                    t_n = tp.tile([P, hi - lo], FP32, tag="tn")
                    nc.scalar.activation(
                        t_n[:], pn[:], mybir.ActivationFunctionType.Identity,
                        bias=bsg_t[:, u:u + 1], scale=1.0 / SW,
                    )
                    pr = tp.tile([P, hi - lo], FP32, tag="pr")
                    nc.vector.tensor_mul(pr[:], t_n[:], eo_sb[:, u, lo:hi])
                    o = op.tile([P, hi - lo], FP32, tag="o")
                    nc.vector.tensor_add(o[:], pr[:], t_m[u][:, lo:hi])
                    # out rides the sync HWDGE queue (idle by the tail),
                    # avoiding the slow SWDGE end-of-kernel drain.
                    nc.sync.dma_start(outT[u][:, lo:hi], o[:])

    nc.compile()
    return nc


def _get_nc():
    global _cached
    if _cached is None:
        _cached = _build()
    return _cached


def kernel(x, weight_mu, weight_sigma, bias_mu, bias_sigma, eps_in, eps_out,
           _trace=False):
    nc = _get_nc()

    # Host-side layout prep (transposes + dtype casts only; no layer math).
    def to_pkb(a):  # [B, D] -> per-core [P, KT, BL] (partition p holds k*128+p)
        a = np.ascontiguousarray(a.astype(_NBF))
        return [
            np.ascontiguousarray(
                a[c * BL:(c + 1) * BL].T.reshape(KT, P, BL).transpose(1, 0, 2))
            for c in range(N_CORES)
        ]

    xs = to_pkb(x)
    eis = to_pkb(eps_in)
    eos = to_pkb(eps_out)  # same transform, u in place of k

    def w_blocks(w, kt):  # [kt*P, U] -> [UT, P(d within block), kt*P] bf16
        wb = w.astype(_NBF).reshape(kt, P, UT, P).transpose(2, 1, 0, 3)
        return np.ascontiguousarray(wb.reshape(UT, P, kt * P))

    wmu_h = w_blocks(weight_mu, KT)
    wsg_s = (weight_sigma.astype(np.float32) * SW)
    wsgb_h = w_blocks(wsg_s[:KB8 * P], KB8)
    # [NDR*2*P, U] -> [UT, P, NDR, 2, P] fp8 (pair i of DoubleRow pass j
    # holds d = (KB8+2j+i)*128 + p)
    w8 = wsg_s[KB8 * P:].reshape(NDR, 2, P, UT, P).transpose(3, 2, 0, 1, 4)
    wsg8_h = np.ascontiguousarray(
        w8.reshape(UT, P, NDR * 2 * P).astype(_NF8))
    bmu_h = np.ascontiguousarray(bias_mu.astype(np.float32).reshape(UT, P).T)
    bsg_h = np.ascontiguousarray(bias_sigma.astype(np.float32).reshape(UT, P).T)

    in_maps = [
        {
            "xT": xs[c],
            "eiT": eis[c],
            "eoT": eos[c],
            "wmu": wmu_h,
            "wsgb": wsgb_h,
            "wsg8": wsg8_h,
            "bmu": bmu_h,
            "bsg": bsg_h,
        }
        for c in range(N_CORES)
    ]

    res = run_bass_kernel_spmd(nc, in_maps, core_ids=list(range(N_CORES)),
                               trace=_trace)
    kernel.last_result = res

    out = np.empty((B, U), dtype=np.float32)
    for c in range(N_CORES):
        oc = res.results[c]["outT"]  # [UT, P, BL]
        out[c * BL:(c + 1) * BL] = oc.transpose(2, 0, 1).reshape(BL, U)
    return out


# revision 34
# speedup vs baseline: 1.1884x; 1.0018x over previous
"""NoisyNet dense layer (training mode) on 8 TRN2 NeuronCores.

out[b,u] = x @ W_mu + eps_out * ((x*eps_in) @ W_sigma) + bias_mu + bias_sigma*eps_out

Sharding: data-parallel over batch (4096 -> 512 rows/core), weights/biases
replicated. On-device math runs in a transposed layout ([D,B]/[U,B]) so the
contraction dim D lands on SBUF partitions; the host does the (free)
transposes, dtype casts and the final gather.

Two-phase schedule: phase 1 runs all 16 mean-term k-loops in bf16 (PSUM
drains to SBUF through the bias-add ACT op), phase 2 runs the noise-term
k-loops and the elementwise combine. In phase 2, k-tiles KB8..15 of the
contraction run as fp8e4 DoubleRow matmuls (2 k-tiles per pass at
2 MACs/cell/cycle); W_sigma is pre-scaled by 512 on the host (exact in
bf16) so both halves share one PSUM scale, undone by the epilogue ACT's
scale=1/512. Error: fp8 on 10/16 of the noise contraction gives rel err
~1.83e-2 vs the 2e-2 gate (verified exactly against the reference on
host; HW matched the host prediction to 6 digits at the 8/16 split).

Scheduling notes (hard-won against the trace):
- DMA bandwidth is shared roughly per-QUEUE, so the weight stream (sync
  queue) can prefetch deep (bufs=8) without slowing the x stream
  (scalar queue) at all.
- Per-queue DMAs are served in issue order, so eps_in/eps_out queued
  behind x can never delay x; issuing them unconditionally (never paced
  on compute progress) keeps the shared DMA-semaphore rotation free of
  cross-queue dependency cycles with the weight stream.
- Phase-1 PSUM drains run on the vector engine: during phase 1 the
  scalar queue is a wall of DMA issues whose semaphore-rotation guards
  resolve only at transfer pace, and an ACT queued behind them stalls
  the PE when the PSUM pool wraps.  The z-production multiplies are
  emitted into the phase-1 loop after their eps_in pieces land so they
  never block the vector queue ahead of those drains.
- u=0..2 k-loops interleave over arriving x chunks so the PE does
  triple work during the DMA-bound start window; warm-up matmuls on
  zeroed SBUF flip the HAM clock gate to 2.4 GHz before real work.
- Outputs ride the sync HWDGE queue (idle by the tail); gpsimd SWDGE
  carries only the tiny biases (mid-kernel block-boundary DRAINs flush
  SWDGE, so nothing long-lived may be outstanding there).
"""

import numpy as np
import ml_dtypes

import concourse.bacc as bacc
import concourse.mybir as mybir
import concourse.tile as tile
from concourse.bass_utils import run_bass_kernel_spmd

N_CORES = 8
B, D, U = 4096, 2048, 2048
BL = B // N_CORES          # 512 batch rows per core
P = 128                    # partitions
KT = D // P                # 16 contraction tiles
UT = U // P                # 16 output tiles
KC = 4                     # k-tiles per activation DMA chunk
NCH = KT // KC             # 4 chunks
KB8 = 6                    # noise-term k-tiles done in bf16 (k 0..5)
NDR = (KT - KB8) // 2      # fp8 DoubleRow passes (k 6..15 as 5 pairs)
SW = 512.0                 # weight scale for the noise matmul (exact pow2)
WARM_MMS = 8               # warm-up matmuls to ride out the DMA ramp
WARM_N = 512               # free dim of warm-up matmuls
BF16 = mybir.dt.bfloat16
FP32 = mybir.dt.float32
FP8 = mybir.dt.float8e4

_NBF = ml_dtypes.bfloat16
_NF8 = ml_dtypes.float8_e4m3

_cached = None


def _build():
    nc = bacc.Bacc("TRN2", target_bir_lowering=False, debug=False)

    # activations laid out [P, KT, BL]: partition p holds d = k*128+p
    xT = nc.declare_dram_parameter("xT", [P, KT, BL], BF16, isOutput=False)
    eiT = nc.declare_dram_parameter("eiT", [P, KT, BL], BF16, isOutput=False)
    eoT = nc.declare_dram_parameter("eoT", [P, UT, BL], BF16, isOutput=False)
    wmu = nc.declare_dram_parameter("wmu", [UT, P, KT * P], BF16, isOutput=False)
    wsgb = nc.declare_dram_parameter("wsgb", [UT, P, KB8 * P], BF16, isOutput=False)
    wsg8 = nc.declare_dram_parameter("wsg8", [UT, P, NDR * 2 * P], FP8, isOutput=False)
    bmu = nc.declare_dram_parameter("bmu", [P, UT], FP32, isOutput=False)
    bsg = nc.declare_dram_parameter("bsg", [P, UT], FP32, isOutput=False)
    outT = nc.declare_dram_parameter("outT", [UT, P, BL], FP32, isOutput=True)

    with tile.TileContext(nc) as tc:
        with (
            tc.tile_pool(name="acts", bufs=1) as acts,
            tc.tile_pool(name="w", bufs=8) as wp,
            tc.tile_pool(name="ws", bufs=4) as wsp,
            tc.tile_pool(name="bias", bufs=1) as bp,
            tc.tile_pool(name="psum", bufs=4, space="PSUM") as pp,
            tc.tile_pool(name="psumn", bufs=3, space="PSUM") as ppn,
            tc.tile_pool(name="mean", bufs=UT) as mp,
            tc.tile_pool(name="tmp", bufs=2) as tp,
            tc.tile_pool(name="out", bufs=3) as op,
        ):
            # HAM warm-up: matmuls on zeroed SBUF during the initial DMA wait
            # so the real matmuls run at 2.4 GHz from the start.
            warm_in = bp.tile([P, BL], BF16, tag="warmin")
            nc.gpsimd.memset(warm_in[:], 0.0)
            warm_ps = ppn.tile([P, BL], FP32, tag="psn")
            for _ in range(WARM_MMS):
                nc.tensor.matmul(warm_ps[:, :WARM_N], warm_in[:, :P],
                                 warm_in[:, :WARM_N])

            # Weight stream (sync queue): all W_mu first, then all W_sigma.
            # Early fetches split so the first matmuls can start sooner.
            wm_tiles = {}
            ws_tiles = {}

            def fetch_wm(u, split=False):
                wm = wp.tile([P, KT * P], BF16, tag="wm")
                if split:
                    nc.sync.dma_start(wm[:, :KC * P], wmu[u][:, :KC * P])
                    nc.sync.dma_start(wm[:, KC * P:], wmu[u][:, KC * P:])
                else:
                    nc.sync.dma_start(wm[:], wmu[u])
                wm_tiles[u] = wm

            def fetch_ws(u):
                wsb_t = wsp.tile([P, KB8 * P], BF16, tag="wsb")
                nc.sync.dma_start(wsb_t[:], wsgb[u])
                ws8_t = wsp.tile([P, NDR, 2, P], FP8, tag="ws8")
                nc.sync.dma_start(ws8_t[:], wsg8[u])
                ws_tiles[u] = (wsb_t, ws8_t)

            fetch_wm(0, split=True)
            fetch_wm(1, split=True)
            fetch_wm(2, split=True)
            fetch_wm(3)

            # x stream (scalar queue) gets the full early DMA bandwidth;
            # eps_in / eps_out issues are paced into the phase-1 loop below.
            x_sb = acts.tile([P, KT, BL], BF16, tag="x")
            ei_sb = acts.tile([P, KT, BL], BF16, tag="ei")
            zb_sb = acts.tile([P, KB8, BL], BF16, tag="zb")
            z8_sb = acts.tile([P, NDR, 2, BL], FP8, tag="z8")
            eo_sb = acts.tile([P, UT, BL], BF16, tag="eo")

            # The scalar queue serves its DMAs in issue order, so eps_in /
            # eps_out queued behind x can never slow the x stream down; and
            # issuing them unconditionally (not paced on compute progress)
            # keeps the shared DMA-semaphore rotation free of cross-queue
            # dependency cycles with the weight stream.
            nc.scalar.dma_start(x_sb[:, 0:1, :], xT[:, 0:1, :])
            nc.scalar.dma_start(x_sb[:, 1:KC, :], xT[:, 1:KC, :])
            for c in range(1, NCH):
                s = slice(c * KC, (c + 1) * KC)
                nc.scalar.dma_start(x_sb[:, s, :], xT[:, s, :])
            for p in range(KT // 2):
                s = slice(2 * p, 2 * p + 2)
                nc.scalar.dma_start(ei_sb[:, s, :], eiT[:, s, :])
            for p in range(UT // 2):
                s = slice(2 * p, 2 * p + 2)
                nc.scalar.dma_start(eo_sb[:, s, :], eoT[:, s, :])

            def z_mult(p):
                # z production for ei piece p; emitted into the phase-1 loop
                # AFTER that piece has landed, so it never blocks the vector
                # queue's FIFO ahead of the mean-term PSUM drains.
                s = slice(2 * p, 2 * p + 2)
                if 2 * p < KB8:
                    nc.vector.tensor_mul(zb_sb[:, s, :], x_sb[:, s, :],
                                         ei_sb[:, s, :])
                else:
                    j = (2 * p - KB8) // 2
                    nc.vector.tensor_mul(z8_sb[:, j], x_sb[:, s, :],
                                         ei_sb[:, s, :])

            # biases (tiny) on the gpsimd SWDGE queue, early.
            bmu_t = bp.tile([P, UT], FP32, tag="bmu")
            nc.gpsimd.dma_start(bmu_t[:], bmu[:])
            bsg_t = bp.tile([P, UT], FP32, tag="bsg")
            nc.gpsimd.dma_start(bsg_t[:], bsg[:])

            # ---- Phase 1: mean terms. t_m[u] = W_mu[u].T @ x + bias_mu[u] ----
            t_m = []

            def drain_mean(u, pm):
                # drain on the vector engine: during phase 1 the scalar queue
                # is a wall of DMA issues whose semaphore-rotation guards
                # resolve only at transfer pace — an ACT behind them would
                # stall the PE when the PSUM pool wraps.
                tm = mp.tile([P, BL], FP32, tag="tm", name=f"tm{u}")
                nc.vector.tensor_scalar_add(tm[:], pm[:], bmu_t[:, u:u + 1])
                t_m.append(tm)

            # u=0..2 interleaved over arriving x chunks: the PE does three
            # k-loops chunk-by-chunk while x streams in (u2 lags one chunk
            # since its weights arrive behind wm0/wm1 on the sync queue).
            pms = [pp.tile([P, BL], FP32, tag="psm", name=f"pm{i}")
                   for i in range(3)]
            wms = [wm_tiles.pop(u) for u in range(3)]

            def ileave_mm(u, k):
                nc.tensor.matmul(
                    pms[u][:], wms[u][:, k * P:(k + 1) * P], x_sb[:, k, :],
                    start=(k == 0), stop=(k == KT - 1),
                )

            for c in range(NCH):
                for u in (0, 1):
                    for k in range(c * KC, (c + 1) * KC):
                        ileave_mm(u, k)
                if c >= 1:
                    for k in range((c - 1) * KC, c * KC):
                        ileave_mm(2, k)
            for k in range((NCH - 1) * KC, KT):
                ileave_mm(2, k)
            for u in range(3):
                drain_mean(u, pms[u])

            for u in range(3, UT):
                if u + 1 < UT:
                    fetch_wm(u + 1)
                if u >= UT - 3:
                    fetch_ws(u - (UT - 3))
                wm_t = wm_tiles.pop(u)
                pm = pp.tile([P, BL], FP32, tag="psm")
                for k in range(KT):
                    nc.tensor.matmul(
                        pm[:], wm_t[:, k * P:(k + 1) * P], x_sb[:, k, :],
                        start=(k == 0), stop=(k == KT - 1),
                    )
                drain_mean(u, pm)
                if 3 <= u <= 10:
                    z_mult(u - 3)

            # ---- Phase 2: noise terms + combine ----
            # PSUM holds 512*noise (both weight halves pre-scaled by 512);
            # the ACT drain applies scale=1/512 and the bias_sigma add.
            for u in range(UT):
                un = u + 3
                if 3 <= un < UT:
                    fetch_ws(un)
                wsb_t, ws8_t = ws_tiles.pop(u)
                # last tiles: split batch so the epilogue pipelines with the
                # final matmuls instead of serializing after them.
                # only the very last tile: DR LDWEIGHTS (~214ns) un-hides
                # below N=512, so halving costs PE time — worth it only where
                # it pipelines the final epilogue with the last matmuls.
                halves = (0, BL // 2, BL) if u == UT - 1 else (0, BL)
                for h in range(len(halves) - 1):
                    lo, hi = halves[h], halves[h + 1]
                    pn = ppn.tile([P, hi - lo], FP32, tag="psn")
                    for k in range(KB8):
                        nc.tensor.matmul(
                            pn[:], wsb_t[:, k * P:(k + 1) * P], zb_sb[:, k, lo:hi],
                            start=(k == 0), stop=False,
                        )
                    for j in range(NDR):
                        nc.tensor.matmul(
                            pn[:], ws8_t[:, j], z8_sb[:, j, :, lo:hi],
                            start=False, stop=(j == NDR - 1),
                            perf_mode=mybir.MatmulPerfMode.DoubleRow,
                        )
                    t_n = tp.tile([P, hi - lo], FP32, tag="tn")
                    nc.scalar.activation(
                        t_n[:], pn[:], mybir.ActivationFunctionType.Identity,
                        bias=bsg_t[:, u:u + 1], scale=1.0 / SW,
                    )
                    pr = tp.tile([P, hi - lo], FP32, tag="pr")
                    nc.vector.tensor_mul(pr[:], t_n[:], eo_sb[:, u, lo:hi])
                    o = op.tile([P, hi - lo], FP32, tag="o")
                    nc.vector.tensor_add(o[:], pr[:], t_m[u][:, lo:hi])
                    # out rides the sync HWDGE queue (idle by the tail),
                    # avoiding the slow SWDGE end-of-kernel drain.
                    nc.sync.dma_start(outT[u][:, lo:hi], o[:])

    nc.compile()
    return nc


def _get_nc():
    global _cached
    if _cached is None:
        _cached = _build()
    return _cached


def kernel(x, weight_mu, weight_sigma, bias_mu, bias_sigma, eps_in, eps_out,
           _trace=False):
    nc = _get_nc()

    # Host-side layout prep (transposes + dtype casts only; no layer math).
    def to_pkb(a):  # [B, D] -> per-core [P, KT, BL] (partition p holds k*128+p)
        a = np.ascontiguousarray(a.astype(_NBF))
        return [
            np.ascontiguousarray(
                a[c * BL:(c + 1) * BL].T.reshape(KT, P, BL).transpose(1, 0, 2))
            for c in range(N_CORES)
        ]

    xs = to_pkb(x)
    eis = to_pkb(eps_in)
    eos = to_pkb(eps_out)  # same transform, u in place of k

    def w_blocks(w, kt):  # [kt*P, U] -> [UT, P(d within block), kt*P] bf16
        wb = w.astype(_NBF).reshape(kt, P, UT, P).transpose(2, 1, 0, 3)
        return np.ascontiguousarray(wb.reshape(UT, P, kt * P))

    wmu_h = w_blocks(weight_mu, KT)
    wsg_s = (weight_sigma.astype(np.float32) * SW)
    wsgb_h = w_blocks(wsg_s[:KB8 * P], KB8)
    # [NDR*2*P, U] -> [UT, P, NDR, 2, P] fp8 (pair i of DoubleRow pass j
    # holds d = (KB8+2j+i)*128 + p)
    w8 = wsg_s[KB8 * P:].reshape(NDR, 2, P, UT, P).transpose(3, 2, 0, 1, 4)
    wsg8_h = np.ascontiguousarray(
        w8.reshape(UT, P, NDR * 2 * P).astype(_NF8))
    bmu_h = np.ascontiguousarray(bias_mu.astype(np.float32).reshape(UT, P).T)
    bsg_h = np.ascontiguousarray(bias_sigma.astype(np.float32).reshape(UT, P).T)

    in_maps = [
        {
            "xT": xs[c],
            "eiT": eis[c],
            "eoT": eos[c],
            "wmu": wmu_h,
            "wsgb": wsgb_h,
            "wsg8": wsg8_h,
            "bmu": bmu_h,
            "bsg": bsg_h,
        }
        for c in range(N_CORES)
    ]

    res = run_bass_kernel_spmd(nc, in_maps, core_ids=list(range(N_CORES)),
                               trace=_trace)
    kernel.last_result = res

    out = np.empty((B, U), dtype=np.float32)
    for c in range(N_CORES):
        oc = res.results[c]["outT"]  # [UT, P, BL]
        out[c * BL:(c + 1) * BL] = oc.transpose(2, 0, 1).reshape(BL, U)
    return out
